# revision 1
# baseline (speedup 1.0000x reference)
"""Chamfer distance loss kernel for Trainium2 (8 NeuronCores, SPMD).

Math: for each batch m, M[i,j] = |t_i|^2 + |s_j|^2 - 2 t_i.s_j  (squared dists)
  dist1 = mean_j sqrt(min_i M), dist2 = mean_i sqrt(min_j M), out = (d1+d2)/2.

Mapping:
  - Data-parallel over the batch dim: 16 batches -> 2 per core.
  - Host packs split-fp16 augmented matrices so one K=15 fp16 matmul emits M
    at ~fp32 precision (hi/lo split of each coordinate and of the squared
    norms; the 16 cross products accumulate in the fp32 PSUM):
      M[i,j] = sum_k L[k,i] * R[k,j]
  - DUAL EMISSION: the PE (otherwise idle) emits both M (i on partitions)
    and M^T (j on partitions, by swapping the matmul operands). Both the
    row-min and the col-min then become elementwise tensor_tensor min
    accumulations across tiles, which run in the DVE 2x bf16 perf mode --
    measured 2x faster than any free-axis reduce op (those are all 1x).
  - ACT drains PSUM -> SBUF bf16 stage tiles (a slice of drains go to DVE
    to balance engine load).
  - The two [128, 4096] accumulators are partition-axis min-reduced via PE
    transpose (8 blocks per PSUM bank) + one 3D-AP reduce per 8 blocks.
  - Device returns raw per-point min squared distances; host does
    sqrt + mean in float64.
"""

import numpy as np

M_BATCH = 16
N = 4096
D = 3
N_CORES = 8
NB = M_BATCH // N_CORES  # batches per core
P = 128
IT = N // P  # 32 tiles per side
K_AUG = 15

# every DVE_DRAIN_MOD-th PSUM chunk is drained by DVE instead of ACT (0=off)
DVE_DRAIN_MOD = 10

# in-kernel repetition count (measurement only; 1 for production)
LOOP_REPS = 1

_CACHE = {}


def _build_nc():
    import concourse.bacc as bacc
    import concourse.tile as tile
    from concourse import mybir
    from concourse.masks import make_identity
    from contextlib import ExitStack, nullcontext

    F32 = mybir.dt.float32
    BF16 = mybir.dt.bfloat16
    FP16 = mybir.dt.float16
    X = mybir.AxisListType.X
    MIN = mybir.AluOpType.min

    JW = 1024  # psum tile width (2 banks)
    JC = N // JW  # 4 psum tiles per emitted tile

    nc = bacc.Bacc("TRN2", target_bir_lowering=False)
    lhsT_d = nc.declare_dram_parameter("lhsT", [NB, K_AUG, N], FP16, isOutput=False)
    rhs_d = nc.declare_dram_parameter("rhs", [NB, K_AUG, N], FP16, isOutput=False)
    mins_d = nc.declare_dram_parameter("mins", [NB, 2, P, IT], F32, isOutput=True)

    with ExitStack() as ctx:
        tc = ctx.enter_context(tile.TileContext(nc))
        consts = ctx.enter_context(tc.tile_pool(name="consts", bufs=1))
        inputs = ctx.enter_context(tc.tile_pool(name="inputs", bufs=2))
        stages = ctx.enter_context(tc.tile_pool(name="stages", bufs=4))
        accs = ctx.enter_context(tc.tile_pool(name="accs", bufs=2))
        outs = ctx.enter_context(tc.tile_pool(name="outs", bufs=2))
        psum = ctx.enter_context(tc.tile_pool(name="psum", bufs=3, space="PSUM"))
        tpsum = ctx.enter_context(tc.tile_pool(name="tpsum", bufs=2, space="PSUM"))

        ident = consts.tile([P, P], BF16)
        make_identity(nc, ident)

        chunk_counter = [0]

        loop_ctx = tc.For_i(0, LOOP_REPS, 1) if LOOP_REPS > 1 else nullcontext()
        with loop_ctx:
          for b in range(NB):
            lhsT_s = inputs.tile([K_AUG, N], FP16, tag="lhsT")
            rhs_s = inputs.tile([K_AUG, N], FP16, tag="rhs")
            nc.sync.dma_start(out=lhsT_s, in_=lhsT_d[b])
            nc.sync.dma_start(out=rhs_s, in_=rhs_d[b])

            # accB[:, 0, j] accumulates min over i-tiles of M[i, j]
            # accB[:, 1, i] accumulates min over j-tiles of M^T[j, i]
            # one combined tensor so each accumulate is a single 8192-wide
            # DVE op (amortizes the per-op bubble)
            accB = accs.tile([P, 2, N], BF16, tag="accB")
            colmins = outs.tile([P, IT], F32, tag="colmins")
            rowmins = outs.tile([P, IT], F32, tag="rowmins")

            for t in range(IT):
                stage = stages.tile([P, 2, N], BF16, tag="stage")
                for side in range(2):
                    if side == 0:
                        w_ap, m_ap = lhsT_s, rhs_s
                    else:
                        w_ap, m_ap = rhs_s, lhsT_s
                    for q in range(JC):
                        ps = psum.tile([P, JW], F32, tag="mm")
                        for h in range(JW // 512):
                            nc.tensor.matmul(
                                ps[:, h * 512 : (h + 1) * 512],
                                w_ap[:, t * P : (t + 1) * P],
                                m_ap[:, q * JW + h * 512 : q * JW + (h + 1) * 512],
                                start=True,
                                stop=True,
                            )
                        # drain PSUM -> SBUF bf16 (ACT mostly, DVE a slice)
                        k = chunk_counter[0]
                        chunk_counter[0] += 1
                        if DVE_DRAIN_MOD and k % DVE_DRAIN_MOD == 0:
                            nc.vector.tensor_copy(
                                out=stage[:, side, q * JW : (q + 1) * JW], in_=ps
                            )
                        else:
                            nc.scalar.copy(
                                out=stage[:, side, q * JW : (q + 1) * JW], in_=ps
                            )
                # both sides' elementwise min accumulate in one DVE 2x op
                if t == 0:
                    nc.vector.tensor_copy(out=accB, in_=stage)
                else:
                    nc.vector.tensor_tensor(accB, stage, accB, MIN)

            # partition-axis min of both accumulators: PE-transpose 128x128
            # blocks, 8 at a time into one PSUM bank, one 3D reduce per group.
            for acc, dest in ((accB[:, 0, :], colmins), (accB[:, 1, :], rowmins)):
                for c8 in range(IT // 8):
                    tp = tpsum.tile([P, 8, P], BF16, tag="tp")
                    for k in range(8):
                        nc.tensor.transpose(
                            tp[:, k, :],
                            acc[:, (c8 * 8 + k) * P : (c8 * 8 + k + 1) * P],
                            ident,
                        )
                    nc.vector.tensor_reduce(
                        out=dest[:, c8 * 8 : (c8 + 1) * 8], in_=tp, axis=X, op=MIN
                    )

            nc.sync.dma_start(out=mins_d[b, 0], in_=colmins)
            nc.sync.dma_start(out=mins_d[b, 1], in_=rowmins)

    nc.compile()
    return nc


def _get_nc():
    if "nc" not in _CACHE:
        _CACHE["nc"] = _build_nc()
    return _CACHE["nc"]


def _prep_inputs(template, source):
    """Build split-fp16 augmented [m, 15, n] operands.

    fp16 two-level split of each coordinate (22-bit effective mantissa) and
    three-level split of the squared norms. Power-of-2 scale balancing keeps
    every stored fp16 value in the normal range; scales cancel exactly in
    each row's product. Row pairing (L_k, R_k):
      per coord c: (-2ah, bh), (-2ah/32, 32*bl), (-128*al, bh/64)  -> 9 rows
      (a2h, 1), (32*a2m, 1/32), (2048*a2l, 1/2048),
      (1, b2h), (1/32, 32*b2m), (1/2048, 2048*b2l)               -> 6 rows
    """
    t = np.ascontiguousarray(template, dtype=np.float32)
    s = np.ascontiguousarray(source, dtype=np.float32)

    f16 = np.float16

    def split2(x):
        h = x.astype(f16).astype(np.float32)
        l = (x - h).astype(f16).astype(np.float32)
        return h, l

    def split3(x):
        h = x.astype(f16).astype(np.float32)
        r = x - h
        m = r.astype(f16).astype(np.float32)
        l = (r - m).astype(f16).astype(np.float32)
        return h, m, l

    ah, al = split2(t)  # [m, n, 3]
    bh, bl = split2(s)
    a2 = (t.astype(np.float64) ** 2).sum(-1).astype(np.float32)  # [m, n]
    b2 = (s.astype(np.float64) ** 2).sum(-1).astype(np.float32)
    a2h, a2m, a2l = split3(a2)
    b2h, b2m, b2l = split3(b2)
    ones = np.ones_like(a2)

    lrows = []
    rrows = []
    for c in range(3):
        lrows += [-2.0 * ah[..., c], (-2.0 / 32.0) * ah[..., c], -128.0 * al[..., c]]
        rrows += [bh[..., c], 32.0 * bl[..., c], bh[..., c] / 64.0]
    lrows += [a2h, 32.0 * a2m, 2048.0 * a2l, ones, ones / 32.0, ones / 2048.0]
    rrows += [ones, ones / 32.0, ones / 2048.0, b2h, 32.0 * b2m, 2048.0 * b2l]

    lhsT = np.stack(lrows, axis=1).astype(f16)  # [m, 15, n]
    rhs = np.stack(rrows, axis=1).astype(f16)
    return np.ascontiguousarray(lhsT), np.ascontiguousarray(rhs)


def run(template, source, trace=False):
    """Returns (result_scalar, exec_time_ns_or_None)."""
    from concourse import bass_utils

    nc = _get_nc()
    lhsT, rhs = _prep_inputs(template, source)
    in_maps = [
        {
            "lhsT": np.ascontiguousarray(lhsT[c * NB : (c + 1) * NB]),
            "rhs": np.ascontiguousarray(rhs[c * NB : (c + 1) * NB]),
        }
        for c in range(N_CORES)
    ]
    res = bass_utils.run_bass_kernel_spmd(
        nc, in_maps, core_ids=list(range(N_CORES)), trace=trace
    )
    mins = np.stack([r["mins"] for r in res.results])  # [8, NB, 2, P, IT]
    total = np.sqrt(np.maximum(mins.astype(np.float64), 0.0)).sum()
    out = np.float32(total / (2.0 * M_BATCH * N))
    return out, res.exec_time_ns


def kernel(template, source):
    out, _ = run(template, source, trace=False)
    return out

